# revision 26
# baseline (speedup 1.0000x reference)
"""Multi-head attention (RoPE, causal, fp32) on 8 Trainium2 NeuronCores.

Problem: B=2, S=2048, D=2048, H=16 heads (hd=128).
Sharding: DP=2 (batch) x TP=4 (head groups of 4 heads). Core c handles
batch c//4, head group c%4. Each core computes q/k/v projections for its
512 features, RoPE, causal attention, and a partial o_proj against its
512 columns of Wo. The host sums the 4 partial o_proj outputs per batch.

Kernel layout strategy (per core):
  - qT, kT in [hd, seq] ("transposed") layout straight out of the
    projection matmuls; v in natural [seq, feat] layout. RoPE applied in
    place at eviction time (rowswap via SBUF->SBUF DMA, sign baked into
    the host-provided sin table).
  - Attention entirely in transposed space: scoresT[k, q] tiles from
    lhsT=kT slice, rhs=qT chunk, N=512. exp fused into the PSUM
    eviction on ScalarE (scale=1/sqrt(hd)), software-pipelined with the
    denominator / attn@V accumulation matmuls two steps behind.
    Causal: only j <= q k-tiles are computed; on the diagonal tile the
    fully-masked 128-wide sub-blocks are zeroed and a single shared
    [128,128] triangular mask is multiplied in.
    Softmax denominator via an all-ones [128,128] stationary matmul
    (yields the k-sum pre-broadcast across partitions); 1/denom via
    4 split VectorE reciprocals; normalization folded into the attn@V
    PSUM eviction.
  - o_proj is weight-stationary and emits the partial TRANSPOSED
    ([D_out, S]); the host transposes back while summing the 4 per-batch
    partials.
All matmuls run as float32r (fp32_mode=HIGH single-pass: full column
rate on fp32 data, ~1e-4-grade precision). Producers of f32r-consumed
SBUF data must write f32r-rounded outputs (BIR verifier requirement).
"""

import sys

for _p in ("/opt/trn_rl_repo",):
    if _p not in sys.path:
        sys.path.insert(0, _p)

import numpy as np

import concourse.bass as bass
import concourse.mybir as mybir
import concourse.tile as tile
from concourse import bacc, bass_utils


def _enable_ldw_opt():
    """walrus ships with --enable-ldw-opt=false; turning it on lets codegen
    elide weight reloads for consecutive matmuls sharing a stationary
    operand (the o_proj and denominator matmuls rely on this)."""
    if getattr(bass_utils, "_ldw_opt_patched", False):
        return
    orig = bass_utils.run_command

    def patched(argv, **kw):
        argv = ["--enable-ldw-opt=true" if a == "--enable-ldw-opt=false" else a
                for a in argv]
        return orig(argv, **kw)

    bass_utils.run_command = patched
    bass_utils._ldw_opt_patched = True


# NOTE: ldw-opt stays OFF: walrus's CoreV3 codegen crashes on bf16
# LDWEIGHTS when --enable-ldw-opt=true, and the bf16 attention path is
# worth more than the elision (weight loads shadow-hide under the 512-wide
# matmuls anyway).
# _enable_ldw_opt()

P = 128          # partitions / head dim
S = 2048         # sequence length
D = 2048         # model dim
F = 512          # features per core (4 heads)
H = 4            # heads per core
HD = 128         # head dim
NJ = D // P      # 16 contraction chunks of 128
NQ = S // 512    # 4 query chunks of 512
SCALE = 1.0 / float(np.sqrt(HD))

F32 = mybir.dt.float32
F32R = mybir.dt.float32r
BF16 = mybir.dt.bfloat16
AFT = mybir.ActivationFunctionType


def _r(ap):
    """View an fp32 AP as float32r for full-rate PE matmuls."""
    return ap.bitcast(F32R)


def _body(tc, xT, wqT, wkT, wvT, woT, cosT, sinT, mskT, out):
    nc = tc.nc

    # long-lived slabs with hand-managed lifetimes; pools are per-side LIFO
    # stacks, so the q/k/v slabs live on the "left" stack while phase-local
    # pools and the oT slab (which outlives q/k/v) use the default side.
    p_qk = tc.alloc_tile_pool(name="p_qk", bufs=1, side="left")   # phases 1..3
    qT = p_qk.tile([P, H, S], F32)    # [hd, head, seq]
    kT = p_qk.tile([P, H, S], F32)

    # v, the exp'd attention weights, and the final output travel as bf16:
    # same PE matmul rate, but half the ScalarE exp time (the attention
    # pace-setter), half the DVE mask time, and half the output DMA. The
    # rel-err budget (2e-2) dwarfs the ~1e-3 this costs.
    p_v = tc.alloc_tile_pool(name="p_v", bufs=1, side="left")     # phases 1..3
    vN = p_v.tile([P, NJ, F], BF16)  # [:, j, :] = v[j*128:(j+1)*128, :]

    # ---------------- projections: q, k (transposed layout) + RoPE ----
    p_xs = tc.alloc_tile_pool(name="p_xs", bufs=12, side="left")   # phases 1..2
    # cs/rot live on the LEFT stack: when the phase-1 pools close, the right
    # stack must free down to wqk only, so p_wv's loads WAR only against the
    # qk matmuls (not the RoPE tail that reads cos/sin/rot).
    with tc.tile_pool(name="cs", bufs=1, side="left") as cspool, \
         tc.tile_pool(name="wqk", bufs=1) as wpool, \
         tc.tile_pool(name="rot", bufs=1, side="left") as rpool, \
         tc.tile_pool(name="pp", bufs=1, space="PSUM") as pp:
        cos_sb = cspool.tile([P, 1, S], F32)
        sin_sb = cspool.tile([P, 1, S], F32)
        wq_sb = wpool.tile([P, NJ, F], F32R)
        wk_sb = wpool.tile([P, NJ, F], F32R)
        for s in range(NQ):
            pq = [pp.tile([P, 512], F32, name=f"pq{s}_{h}", tag=f"pq{h}")
                  for h in range(H)]
            pk = [pp.tile([P, 512], F32, name=f"pk{s}_{h}", tag=f"pk{h}")
                  for h in range(H)]
            for j in range(NJ):
                xt = p_xs.tile([P, 512], F32R, name=f"xt{s}_{j}", tag="xt")
                # x chunk first: the very first matmul waits on it, the
                # weight chunks follow right behind in the same queues
                nc.sync.dma_start(xt[:], xT[j * P:(j + 1) * P, s * 512:(s + 1) * 512])
                if s == 0:
                    # weight loads interleaved with the first x chunks so
                    # the first matmuls aren't stuck behind 8MB of DMA
                    nc.sync.dma_start(wq_sb[:, j, :], wqT[j * P:(j + 1) * P, :])
                    nc.sync.dma_start(wk_sb[:, j, :], wkT[j * P:(j + 1) * P, :])

                for h in range(H):
                    nc.tensor.matmul(pq[h][:], _r(wq_sb[:, j, h * HD:(h + 1) * HD]),
                                     _r(xt[:]), start=(j == 0), stop=(j == NJ - 1))
                    nc.tensor.matmul(pk[h][:], _r(wk_sb[:, j, h * HD:(h + 1) * HD]),
                                     _r(xt[:]), start=(j == 0), stop=(j == NJ - 1))
            sl = slice(s * 512, (s + 1) * 512)
            # cos/sin arrive piecewise, issued here (between chunks) so the
            # 256KB transfers never queue ahead of the per-j x-tile loads
            nc.sync.dma_start(cos_sb[:, 0, sl], cosT[:, sl])
            nc.sync.dma_start(sin_sb[:, 0, sl], sinT[:, sl])
            # Per head: evict the tile plus a rowswapped copy straight out
            # of PSUM. The swap is done by evicting the two partition halves
            # crosswise (engines can read/write different partition
            # windows) -- no SBUF->SBUF DMAs, so the x/weight loads never
            # queue behind the RoPE tail. Full evictions on ScalarE; the
            # half evictions split ScalarE/VectorE so the 24-op chain drains
            # ~2x faster. pq tags evicted before pk: the v-projection (which
            # reuses the pq banks) can then start after only 4 tags free.
            rts = {}
            for psl, slab, tag in ((pq, qT, "rtq"), (pk, kT, "rtk")):
                rt = rpool.tile([P, H, 512], F32, name=f"rt{s}_{tag}", tag=tag)
                rts[tag] = rt
                for h in range(H):
                    nc.scalar.activation(slab[:, h, sl].bitcast(F32R), psl[h][:],
                                         AFT.Copy)
                    nc.scalar.activation(rt[0:64, h, :], psl[h][64:128, :], AFT.Copy)
                    nc.vector.tensor_copy(rt[64:128, h, :], psl[h][0:64, :])
            # ...then RoPE in place, 4 heads per DVE op:
            # dst = dst*cos + rowswap(dst)*sin (sign baked into sin table)
            for slab, tag in ((qT, "rtq"), (kT, "rtk")):
                rt = rts[tag]
                dst = slab[:, :, sl]
                _, cos_b = bass.broadcast_tensor_aps(dst, cos_sb[:, :, sl])
                _, sin_b = bass.broadcast_tensor_aps(dst, sin_sb[:, :, sl])
                nc.vector.tensor_mul(rt[:], rt[:], sin_b)
                nc.vector.tensor_mul(dst.bitcast(F32R), dst, cos_b)
                nc.vector.tensor_add(dst.bitcast(F32R), dst, rt[:])

        # ------------ projection: v (natural layout), same pools ----------
        # wv reuses the wq_sb slab (its readers -- the chunk-3 matmuls --
        # finish progressively, so these loads prefetch during chunk 3) and
        # the pv accumulators reuse the pq/pk PSUM tags. No new pools means
        # no pool-boundary barrier: the old structure serialized v behind
        # the whole RoPE tail via the PSUM pool alloc.
        for j in range(NJ):
            nc.sync.dma_start(wq_sb[:, j, :], wvT[j * P:(j + 1) * P, :])
        for sg in range(4):
            tg = "pq" if sg % 2 == 0 else "pk"
            pv = [pp.tile([P, F], F32, name=f"pv{sg}_{st}", tag=f"{tg}{st}")
                  for st in range(4)]
            for j in range(NJ):
                xt2 = p_xs.tile([P, 512], F32R, name=f"x2{sg}_{j}", tag="xt")
                nc.sync.dma_start(xt2[:], xT[j * P:(j + 1) * P, sg * 512:(sg + 1) * 512])
                for st in range(4):
                    nc.tensor.matmul(pv[st][:], _r(xt2[:, st * P:(st + 1) * P]),
                                     _r(wq_sb[:, j, :]), start=(j == 0), stop=(j == NJ - 1))
            for st in range(4):
                if st % 2 == 0:
                    nc.scalar.activation(vN[:, sg * 4 + st, :], pv[st][:], AFT.Copy)
                else:
                    nc.vector.tensor_copy(vN[:, sg * 4 + st, :], pv[st][:])

    p_xs.release()

    # ---------------- attention (all in transposed space) -------------
    # attention-phase SBUF lives on the RIGHT stack so nothing here reuses
    # the just-released wv/xs2 space (which would add WAR waits on the tail
    # of the v pass). Wo is prefetched here too, for the same reason plus
    # DMA overlap with attention compute.
    p_oT = tc.alloc_tile_pool(name="p_oT", bufs=1, side="right")  # phases 3..4
    oT = p_oT.tile([P, H, S], F32)    # attention output, transposed
    # Wo prefetched below the attention pools on the right stack so its 4MB
    # of DMA overlaps attention compute and o_proj starts without a stall.
    p_wo = tc.alloc_tile_pool(name="p_wo", bufs=1, side="right")
    wo_sb = p_wo.tile([P, H, D], F32R)
    with tc.tile_pool(name="amsk", bufs=1, side="right") as mpool, \
         tc.tile_pool(name="exp", bufs=1, side="right") as epool, \
         tc.tile_pool(name="attsb", bufs=2, side="right") as apool, \
         tc.tile_pool(name="pa", bufs=1, space="PSUM") as pap:
        msk_sb = mpool.tile([P, P], BF16)
        nc.sync.dma_start(msk_sb[:], mskT)
        for h in range(H):
            nc.sync.dma_start(wo_sb[:, h, :], woT[h * P:(h + 1) * P, :])
        # all-ones [128,128] stationary: the denominator matmul then yields
        # the k-sum already broadcast across all 128 partitions of PSUM.
        ones_tmp = mpool.tile([P, P], BF16)
        nc.vector.memset(ones_tmp[:], 1.0)
        ones_mat = mpool.tile([P, P], BF16)
        nc.vector.tensor_copy(ones_mat[:], ones_tmp[:])

        # One flat software pipeline over all (head, q-chunk, k-pair) work
        # items: the accumulation matmuls lag the score/exp stage by LAG
        # pairs and cross (h,q) boundaries, so the exp latency is never
        # exposed at iteration starts. Within a step the acc is issued
        # BEFORE the next score so reuse of the shared ex buffer is a
        # plain engine-ordered WAR, never a corruption.
        # Causal narrowing: k-tile j only attends queries >= 128*(j-4q), so
        # score/exp/acc all operate on the [128*d, 512) column slice; the
        # fully-masked sub-blocks are never computed at all.
        items = []
        for h in range(H):
            for q in range(NQ):
                jmax = 4 * (q + 1)
                for j in range(0, jmax, 2):
                    items.append((h, q, j, jmax))
        state = {}

        def _lo(q, j):
            # fp32r matmuls below 256 moving-dim run at 1/4 rate, so the
            # f32r score matmuls clamp the causal narrowing at 256 columns
            return max(0, min((j - 4 * q), 2)) * P

        def _alo(q, j):
            # the accumulation matmuls have bf16 rhs (no sub-256 penalty),
            # so they narrow fully to the true causal boundary; the exp'd
            # scores in the clamp slack [256:384) are then simply never
            # read, and no zero-fill is needed
            return max(0, j - 4 * q) * P

        def score_step(it):
            h, q, j, jmax = it
            key = (h, q)
            if key not in state:
                # double-buffered across (h,q) iterations: the next
                # iteration's exp never WAR-waits on this one's last accs
                state[key] = {"ex": epool.tile([P, NJ, 512], BF16,
                                               name=f"ex{h}_{q}",
                                               tag=f"ex{(h * NQ + q) % 2}")}
            ex = state[key]["ex"]
            lo = _lo(q, j)
            psc = pap.tile([P, 2, 512], F32, name=f"psc{h}{q}{j}",
                           tag="psc", bufs=2)
            for t in range(2):
                tlo = _lo(q, j + t)
                nc.tensor.matmul(psc[:, t, tlo:512],
                                 _r(kT[:, h, (j + t) * P:(j + t + 1) * P]),
                                 _r(qT[:, h, q * 512 + tlo:(q + 1) * 512]),
                                 start=True, stop=True)
            nc.scalar.activation(ex[:, j:j + 2, lo:512],
                                 psc[:, :, lo:512], AFT.Exp, scale=SCALE)
            for t in range(2):
                dt = (j + t) - 4 * q
                if dt >= 0:
                    nc.vector.tensor_mul(
                        ex[:, j + t, dt * P:(dt + 1) * P],
                        ex[:, j + t, dt * P:(dt + 1) * P], msk_sb[:])

        def acc_pair(it):
            h, q, j, jmax = it
            st = state[(h, q)]
            ex = st["ex"]
            if "pden" not in st:
                st["pden"] = pap.tile([P, 512], F32, name=f"pden{h}{q}",
                                      tag="pden", bufs=2)
                st["pov"] = pap.tile([P, 512], F32, name=f"pov{h}{q}",
                                     tag="pov", bufs=2)
            pden, pov = st["pden"], st["pov"]
            lo = [_alo(q, j + t) for t in range(2)]
            # both pden matmuls back to back: ldw-opt elides the ones
            # reload on the second
            for t in range(2):
                nc.tensor.matmul(pden[:, lo[t]:512], ones_mat[:],
                                 ex[:, j + t, lo[t]:512],
                                 start=(j + t == 0), stop=(j + t == jmax - 1))
            for t in range(2):
                nc.tensor.matmul(pov[:, lo[t]:512],
                                 vN[:, j + t, h * HD:(h + 1) * HD],
                                 ex[:, j + t, lo[t]:512],
                                 start=(j + t == 0), stop=(j + t == jmax - 1))

        def finish(it):
            h, q, j, jmax = it
            st = state.pop((h, q))
            rbc = apool.tile([P, 512], F32, name=f"rbc{h}{q}", tag="rbc")
            nc.vector.reciprocal_approx_fast(rbc[:], st["pden"][:])
            nc.vector.tensor_mul(oT[:, h, q * 512:(q + 1) * 512].bitcast(F32R),
                                 st["pov"][:], rbc[:])

        LAG = 3
        for i in range(len(items) + LAG):
            k = i - LAG
            if k >= 0:
                acc_pair(items[k])
                if items[k][2] + 2 >= items[k][3]:
                    finish(items[k])
            if i < len(items):
                score_step(items[i])

    p_v.release()
    p_qk.release()

    # ---------------- o_proj (partial against this core's Wo cols) ----
    # weight-stationary: lhsT = Wo chunk reused across all 4 q-chunks.
    # Output is produced TRANSPOSED ([D_out, S]); the host transposes back.
    with tc.tile_pool(name="oev", bufs=4) as oevp, \
         tc.tile_pool(name="po", bufs=1, space="PSUM") as pop:
        for dt in range(D // P):
            po = [pop.tile([P, 512], F32, name=f"po{dt}_{qc}", tag=f"po{qc}", bufs=2)
                  for qc in range(NQ)]
            for h in range(H):
                for qc in range(NQ):
                    nc.tensor.matmul(po[qc][:], _r(wo_sb[:, h, dt * P:(dt + 1) * P]),
                                     _r(oT[:, h, qc * 512:(qc + 1) * 512]),
                                     start=(h == 0), stop=(h == H - 1))
            for qc in range(NQ):
                ot = oevp.tile([P, 512], BF16, name=f"ot{dt}_{qc}", tag="ot")
                if (dt + qc) % 2 == 0:
                    nc.vector.tensor_copy(ot[:], po[qc][:])
                else:
                    nc.scalar.activation(ot[:], po[qc][:], AFT.Copy)
                nc.sync.dma_start(out[dt * P:(dt + 1) * P, qc * 512:(qc + 1) * 512], ot[:])
    p_wo.release()
    p_oT.release()


def build_nc():
    nc = bacc.Bacc("TRN2", target_bir_lowering=False, debug=False,
                   enable_asserts=True, num_devices=8)
    xT = nc.dram_tensor("xT", [D, S], F32R, kind="ExternalInput").ap()
    wqT = nc.dram_tensor("wqT", [D, F], F32R, kind="ExternalInput").ap()
    wkT = nc.dram_tensor("wkT", [D, F], F32R, kind="ExternalInput").ap()
    wvT = nc.dram_tensor("wvT", [D, F], F32R, kind="ExternalInput").ap()
    woT = nc.dram_tensor("woT", [F, D], F32R, kind="ExternalInput").ap()
    cosT = nc.dram_tensor("cosT", [P, S], F32, kind="ExternalInput").ap()
    sinT = nc.dram_tensor("sinT", [P, S], F32, kind="ExternalInput").ap()
    mskT = nc.dram_tensor("mskT", [P, P], BF16, kind="ExternalInput").ap()
    out = nc.dram_tensor("out", [S, D], BF16, kind="ExternalOutput").ap()

    with tile.TileContext(nc) as tc:
        _body(tc, xT, wqT, wkT, wvT, woT, cosT, sinT, mskT, out)
    nc.compile()
    return nc


_CACHE = {}


def _get_nc():
    if "nc" not in _CACHE:
        _CACHE["nc"] = build_nc()
    return _CACHE["nc"]


def _rope_tables():
    hd = HD
    inv = 1.0 / (10000.0 ** (np.arange(0, hd, 2, dtype=np.float32) / np.float32(hd)))
    t = np.arange(S, dtype=np.float32)
    freqs = np.outer(t, inv)                      # [S, 64]
    emb = np.concatenate([freqs, freqs], axis=-1)  # [S, 128]
    cosT = np.cos(emb).T.astype(np.float32).copy()
    sinT = np.sin(emb).T.astype(np.float32).copy()
    sinT[0:64, :] *= -1.0  # sign of rotate_half baked into the table
    return np.ascontiguousarray(cosT), np.ascontiguousarray(sinT)


def _diag_masks():
    import ml_dtypes
    kp = np.arange(P)[:, None]
    qf = np.arange(P)[None, :]
    return np.ascontiguousarray((kp <= qf).astype(ml_dtypes.bfloat16))


def _in_maps(x, Wq, Wk, Wv, Wo):
    cosT, sinT = _rope_tables()
    msk = _diag_masks()
    maps = []
    for c in range(8):
        b, g = c // 4, c % 4
        fs = slice(g * F, (g + 1) * F)
        maps.append({
            "xT": np.ascontiguousarray(x[b].T),
            "wqT": np.ascontiguousarray(Wq[fs, :].T),
            "wkT": np.ascontiguousarray(Wk[fs, :].T),
            "wvT": np.ascontiguousarray(Wv[fs, :].T),
            "woT": np.ascontiguousarray(Wo[:, fs].T),
            "cosT": cosT,
            "sinT": sinT,
            "mskT": msk,
        })
    return maps


def run(x, Wq, Wk, Wv, Wo, trace=False, **spmd_kwargs):
    """Run on 8 cores; returns (full_output, BassKernelResults)."""
    x = np.asarray(x, np.float32)
    Wq = np.asarray(Wq, np.float32)
    Wk = np.asarray(Wk, np.float32)
    Wv = np.asarray(Wv, np.float32)
    Wo = np.asarray(Wo, np.float32)
    nc = _get_nc()
    maps = _in_maps(x, Wq, Wk, Wv, Wo)
    res = bass_utils.run_bass_kernel_spmd(nc, maps, core_ids=list(range(8)),
                                          trace=trace, **spmd_kwargs)
    outs = [np.asarray(res.results[c]["out"], dtype=np.float32) for c in range(8)]
    full = np.empty((2, S, D), np.float32)
    for b in range(2):
        # each core returns its o_proj partial TRANSPOSED ([D_out, S]), bf16
        acc = outs[4 * b] + outs[4 * b + 1] + outs[4 * b + 2] + outs[4 * b + 3]
        full[b] = acc.T
    return full, res


def kernel(x, Wq, Wk, Wv, Wo):
    full, _ = run(x, Wq, Wk, Wv, Wo)
    return full



# revision 27
# speedup vs baseline: 1.1830x; 1.1830x over previous
"""Multi-head attention (RoPE, causal, fp32) on 8 Trainium2 NeuronCores.

Problem: B=2, S=2048, D=2048, H=16 heads (hd=128).
Sharding: DP=2 (batch) x TP=4 (head groups of 4 heads). Core c handles
batch c//4, head group c%4. Each core computes q/k/v projections for its
512 features, RoPE, causal attention, and a partial o_proj against its
512 columns of Wo. The host sums the 4 partial o_proj outputs per batch.

Kernel layout strategy (per core):
  - qT, kT in [hd, seq] ("transposed") layout straight out of the
    projection matmuls; v in natural [seq, feat] layout. RoPE applied in
    place at eviction time (rowswap via SBUF->SBUF DMA, sign baked into
    the host-provided sin table).
  - Attention entirely in transposed space: scoresT[k, q] tiles from
    lhsT=kT slice, rhs=qT chunk, N=512. exp fused into the PSUM
    eviction on ScalarE (scale=1/sqrt(hd)), software-pipelined with the
    denominator / attn@V accumulation matmuls two steps behind.
    Causal: only j <= q k-tiles are computed; on the diagonal tile the
    fully-masked 128-wide sub-blocks are zeroed and a single shared
    [128,128] triangular mask is multiplied in.
    Softmax denominator via an all-ones [128,128] stationary matmul
    (yields the k-sum pre-broadcast across partitions); 1/denom via
    4 split VectorE reciprocals; normalization folded into the attn@V
    PSUM eviction.
  - o_proj is weight-stationary and emits the partial TRANSPOSED
    ([D_out, S]); the host transposes back while summing the 4 per-batch
    partials.
All matmuls run as float32r (fp32_mode=HIGH single-pass: full column
rate on fp32 data, ~1e-4-grade precision). Producers of f32r-consumed
SBUF data must write f32r-rounded outputs (BIR verifier requirement).
"""

import sys

for _p in ("/opt/trn_rl_repo",):
    if _p not in sys.path:
        sys.path.insert(0, _p)

import numpy as np

import concourse.bass as bass
import concourse.mybir as mybir
import concourse.tile as tile
from concourse import bacc, bass_utils


def _enable_ldw_opt():
    """walrus ships with --enable-ldw-opt=false; turning it on lets codegen
    elide weight reloads for consecutive matmuls sharing a stationary
    operand (the o_proj and denominator matmuls rely on this)."""
    if getattr(bass_utils, "_ldw_opt_patched", False):
        return
    orig = bass_utils.run_command

    def patched(argv, **kw):
        argv = ["--enable-ldw-opt=true" if a == "--enable-ldw-opt=false" else a
                for a in argv]
        return orig(argv, **kw)

    bass_utils.run_command = patched
    bass_utils._ldw_opt_patched = True


# NOTE: ldw-opt stays OFF: walrus's CoreV3 codegen crashes on bf16
# LDWEIGHTS when --enable-ldw-opt=true, and the bf16 attention path is
# worth more than the elision (weight loads shadow-hide under the 512-wide
# matmuls anyway).
# _enable_ldw_opt()

P = 128          # partitions / head dim
S = 2048         # sequence length
D = 2048         # model dim
F = 512          # features per core (4 heads)
H = 4            # heads per core
HD = 128         # head dim
NJ = D // P      # 16 contraction chunks of 128
NQ = S // 512    # 4 query chunks of 512
SCALE = 1.0 / float(np.sqrt(HD))

F32 = mybir.dt.float32
F32R = mybir.dt.float32r
BF16 = mybir.dt.bfloat16
AFT = mybir.ActivationFunctionType


def _r(ap):
    """View an fp32 AP as float32r for full-rate PE matmuls."""
    return ap.bitcast(F32R)


def _body(tc, xT, wqT, wkT, wvT, woT, cosT, sinT, mskT, out):
    nc = tc.nc

    # long-lived slabs with hand-managed lifetimes; pools are per-side LIFO
    # stacks, so the q/k/v slabs live on the "left" stack while phase-local
    # pools and the oT slab (which outlives q/k/v) use the default side.
    p_qk = tc.alloc_tile_pool(name="p_qk", bufs=1, side="left")   # phases 1..3
    qT = p_qk.tile([P, H, S], F32)    # [hd, head, seq]
    kT = p_qk.tile([P, H, S], F32)

    # v, the exp'd attention weights, and the final output travel as bf16:
    # same PE matmul rate, but half the ScalarE exp time (the attention
    # pace-setter), half the DVE mask time, and half the output DMA. The
    # rel-err budget (2e-2) dwarfs the ~1e-3 this costs.
    p_v = tc.alloc_tile_pool(name="p_v", bufs=1, side="left")     # phases 1..3
    vN = p_v.tile([P, NJ, F], BF16)  # [:, j, :] = v[j*128:(j+1)*128, :]

    # ---------------- projections: q, k (transposed layout) + RoPE ----
    p_xs = tc.alloc_tile_pool(name="p_xs", bufs=12, side="left")   # phases 1..2
    # cs/rot live on the LEFT stack: when the phase-1 pools close, the right
    # stack must free down to wqk only, so p_wv's loads WAR only against the
    # qk matmuls (not the RoPE tail that reads cos/sin/rot).
    with tc.tile_pool(name="cs", bufs=1, side="left") as cspool, \
         tc.tile_pool(name="wqk", bufs=1) as wpool, \
         tc.tile_pool(name="rot", bufs=1, side="left") as rpool, \
         tc.tile_pool(name="pp", bufs=1, space="PSUM") as pp:
        cos_sb = cspool.tile([P, 1, S], F32)
        sin_sb = cspool.tile([P, 1, S], F32)
        wq_sb = wpool.tile([P, NJ, F], F32R)
        wk_sb = wpool.tile([P, NJ, F], F32R)
        for s in range(NQ):
            pq = [pp.tile([P, 512], F32, name=f"pq{s}_{h}", tag=f"pq{h}")
                  for h in range(H)]
            pk = [pp.tile([P, 512], F32, name=f"pk{s}_{h}", tag=f"pk{h}")
                  for h in range(H)]
            for j in range(NJ):
                xt = p_xs.tile([P, 512], F32R, name=f"xt{s}_{j}", tag="xt")
                # x chunk first: the very first matmul waits on it, the
                # weight chunks follow right behind in the same queues
                nc.sync.dma_start(xt[:], xT[j * P:(j + 1) * P, s * 512:(s + 1) * 512])
                if s == 0:
                    # weight loads interleaved with the first x chunks so
                    # the first matmuls aren't stuck behind 8MB of DMA
                    nc.sync.dma_start(wq_sb[:, j, :], wqT[j * P:(j + 1) * P, :])
                    nc.sync.dma_start(wk_sb[:, j, :], wkT[j * P:(j + 1) * P, :])

                for h in range(H):
                    nc.tensor.matmul(pq[h][:], _r(wq_sb[:, j, h * HD:(h + 1) * HD]),
                                     _r(xt[:]), start=(j == 0), stop=(j == NJ - 1))
                    nc.tensor.matmul(pk[h][:], _r(wk_sb[:, j, h * HD:(h + 1) * HD]),
                                     _r(xt[:]), start=(j == 0), stop=(j == NJ - 1))
            sl = slice(s * 512, (s + 1) * 512)
            # cos/sin arrive piecewise, issued here (between chunks) so the
            # 256KB transfers never queue ahead of the per-j x-tile loads
            nc.sync.dma_start(cos_sb[:, 0, sl], cosT[:, sl])
            nc.sync.dma_start(sin_sb[:, 0, sl], sinT[:, sl])
            # Per head: evict the tile plus a rowswapped copy straight out
            # of PSUM. The swap is done by evicting the two partition halves
            # crosswise (engines can read/write different partition
            # windows) -- no SBUF->SBUF DMAs, so the x/weight loads never
            # queue behind the RoPE tail. Full evictions on ScalarE; the
            # half evictions split ScalarE/VectorE so the 24-op chain drains
            # ~2x faster. pq tags evicted before pk: the v-projection (which
            # reuses the pq banks) can then start after only 4 tags free.
            rts = {}
            for psl, slab, tag in ((pq, qT, "rtq"), (pk, kT, "rtk")):
                rt = rpool.tile([P, H, 512], F32, name=f"rt{s}_{tag}", tag=tag)
                rts[tag] = rt
                for h in range(H):
                    nc.scalar.activation(slab[:, h, sl].bitcast(F32R), psl[h][:],
                                         AFT.Copy)
                    nc.scalar.activation(rt[0:64, h, :], psl[h][64:128, :], AFT.Copy)
                    nc.vector.tensor_copy(rt[64:128, h, :], psl[h][0:64, :])
            # ...then RoPE in place, 4 heads per DVE op:
            # dst = dst*cos + rowswap(dst)*sin (sign baked into sin table)
            for slab, tag in ((qT, "rtq"), (kT, "rtk")):
                rt = rts[tag]
                dst = slab[:, :, sl]
                _, cos_b = bass.broadcast_tensor_aps(dst, cos_sb[:, :, sl])
                _, sin_b = bass.broadcast_tensor_aps(dst, sin_sb[:, :, sl])
                nc.vector.tensor_mul(rt[:], rt[:], sin_b)
                nc.vector.tensor_mul(dst.bitcast(F32R), dst, cos_b)
                nc.vector.tensor_add(dst.bitcast(F32R), dst, rt[:])

        # ------------ projection: v (natural layout), same pools ----------
        # wv reuses the wq_sb slab (its readers -- the chunk-3 matmuls --
        # finish progressively, so these loads prefetch during chunk 3) and
        # the pv accumulators reuse the pq/pk PSUM tags. No new pools means
        # no pool-boundary barrier: the old structure serialized v behind
        # the whole RoPE tail via the PSUM pool alloc.
        for j in range(NJ):
            nc.sync.dma_start(wq_sb[:, j, :], wvT[j * P:(j + 1) * P, :])
        for sg in range(4):
            tg = "pq" if sg % 2 == 0 else "pk"
            pv = [pp.tile([P, F], F32, name=f"pv{sg}_{st}", tag=f"{tg}{st}")
                  for st in range(4)]
            for j in range(NJ):
                xt2 = p_xs.tile([P, 512], F32R, name=f"x2{sg}_{j}", tag="xt")
                nc.sync.dma_start(xt2[:], xT[j * P:(j + 1) * P, sg * 512:(sg + 1) * 512])
                for st in range(4):
                    nc.tensor.matmul(pv[st][:], _r(xt2[:, st * P:(st + 1) * P]),
                                     _r(wq_sb[:, j, :]), start=(j == 0), stop=(j == NJ - 1))
            for st in range(4):
                if st % 2 == 0:
                    nc.scalar.activation(vN[:, sg * 4 + st, :], pv[st][:], AFT.Copy)
                else:
                    nc.vector.tensor_copy(vN[:, sg * 4 + st, :], pv[st][:])

    p_xs.release()

    # ---------------- attention (all in transposed space) -------------
    # attention-phase SBUF lives on the RIGHT stack so nothing here reuses
    # the just-released wv/xs2 space (which would add WAR waits on the tail
    # of the v pass). Wo is prefetched here too, for the same reason plus
    # DMA overlap with attention compute.
    p_oT = tc.alloc_tile_pool(name="p_oT", bufs=1, side="right")  # phases 3..4
    oT = p_oT.tile([P, H, S], F32)    # attention output, transposed
    # Wo prefetched below the attention pools on the right stack so its 4MB
    # of DMA overlaps attention compute and o_proj starts without a stall.
    p_wo = tc.alloc_tile_pool(name="p_wo", bufs=1, side="right")
    wo_sb = p_wo.tile([P, H, D], F32R)
    with tc.tile_pool(name="amsk", bufs=1, side="right") as mpool, \
         tc.tile_pool(name="exp", bufs=1, side="right") as epool, \
         tc.tile_pool(name="attsb", bufs=2, side="right") as apool, \
         tc.tile_pool(name="pa", bufs=1, space="PSUM") as pap:
        msk_sb = mpool.tile([P, P], BF16)
        nc.sync.dma_start(msk_sb[:], mskT)
        for h in range(H):
            nc.sync.dma_start(wo_sb[:, h, :], woT[h * P:(h + 1) * P, :])
        # all-ones [128,128] stationary: the denominator matmul then yields
        # the k-sum already broadcast across all 128 partitions of PSUM.
        ones_tmp = mpool.tile([P, P], BF16)
        nc.vector.memset(ones_tmp[:], 1.0)
        ones_mat = mpool.tile([P, P], BF16)
        nc.vector.tensor_copy(ones_mat[:], ones_tmp[:])

        # One flat software pipeline over all (head, q-chunk, k-pair) work
        # items: the accumulation matmuls lag the score/exp stage by LAG
        # pairs and cross (h,q) boundaries, so the exp latency is never
        # exposed at iteration starts. Within a step the acc is issued
        # BEFORE the next score so reuse of the shared ex buffer is a
        # plain engine-ordered WAR, never a corruption.
        # Causal narrowing: k-tile j only attends queries >= 128*(j-4q), so
        # score/exp/acc all operate on the [128*d, 512) column slice; the
        # fully-masked sub-blocks are never computed at all.
        items = []
        for h in range(H):
            for q in range(NQ):
                jmax = 4 * (q + 1)
                for j in range(0, jmax, 2):
                    items.append((h, q, j, jmax))
        state = {}

        def _lo(q, j):
            # fp32r matmuls below 256 moving-dim run at 1/4 rate, so the
            # f32r score matmuls clamp the causal narrowing at 256 columns
            return max(0, min((j - 4 * q), 2)) * P

        def _alo(q, j):
            # the accumulation matmuls have bf16 rhs (no sub-256 penalty),
            # so they narrow fully to the true causal boundary; the exp'd
            # scores in the clamp slack [256:384) are then simply never
            # read, and no zero-fill is needed
            return max(0, j - 4 * q) * P

        def score_step(it):
            h, q, j, jmax = it
            key = (h, q)
            if key not in state:
                # double-buffered across (h,q) iterations: the next
                # iteration's exp never WAR-waits on this one's last accs
                state[key] = {"ex": epool.tile([P, NJ, 512], BF16,
                                               name=f"ex{h}_{q}",
                                               tag=f"ex{(h * NQ + q) % 2}")}
            ex = state[key]["ex"]
            lo = _lo(q, j)
            psc = pap.tile([P, 2, 512], F32, name=f"psc{h}{q}{j}",
                           tag="psc", bufs=2)
            for t in range(2):
                tlo = _lo(q, j + t)
                nc.tensor.matmul(psc[:, t, tlo:512],
                                 _r(kT[:, h, (j + t) * P:(j + t + 1) * P]),
                                 _r(qT[:, h, q * 512 + tlo:(q + 1) * 512]),
                                 start=True, stop=True)
            nc.scalar.activation(ex[:, j:j + 2, lo:512],
                                 psc[:, :, lo:512], AFT.Exp, scale=SCALE)
            for t in range(2):
                dt = (j + t) - 4 * q
                if dt >= 0:
                    nc.vector.tensor_mul(
                        ex[:, j + t, dt * P:(dt + 1) * P],
                        ex[:, j + t, dt * P:(dt + 1) * P], msk_sb[:])
                if dt == 3:
                    # clamped block [256:384) holds exp of fully-masked
                    # scores; zero it before the accumulation reads it
                    nc.vector.tensor_scalar_mul(
                        ex[:, j + t, 2 * P:3 * P],
                        ex[:, j + t, 2 * P:3 * P], 0.0)

        def acc_pair(it):
            h, q, j, jmax = it
            st = state[(h, q)]
            ex = st["ex"]
            if "pden" not in st:
                st["pden"] = pap.tile([P, 512], F32, name=f"pden{h}{q}",
                                      tag="pden", bufs=2)
                st["pov"] = pap.tile([P, 512], F32, name=f"pov{h}{q}",
                                     tag="pov", bufs=2)
            pden, pov = st["pden"], st["pov"]
            lo = [_lo(q, j + t) for t in range(2)]
            # both pden matmuls back to back: ldw-opt elides the ones
            # reload on the second
            for t in range(2):
                nc.tensor.matmul(pden[:, lo[t]:512], ones_mat[:],
                                 ex[:, j + t, lo[t]:512],
                                 start=(j + t == 0), stop=(j + t == jmax - 1))
            for t in range(2):
                nc.tensor.matmul(pov[:, lo[t]:512],
                                 vN[:, j + t, h * HD:(h + 1) * HD],
                                 ex[:, j + t, lo[t]:512],
                                 start=(j + t == 0), stop=(j + t == jmax - 1))

        def finish(it):
            h, q, j, jmax = it
            st = state.pop((h, q))
            rbc = apool.tile([P, 512], F32, name=f"rbc{h}{q}", tag="rbc")
            nc.vector.reciprocal_approx_fast(rbc[:], st["pden"][:])
            nc.vector.tensor_mul(oT[:, h, q * 512:(q + 1) * 512].bitcast(F32R),
                                 st["pov"][:], rbc[:])

        LAG = 3
        for i in range(len(items) + LAG):
            k = i - LAG
            if k >= 0:
                acc_pair(items[k])
                if items[k][2] + 2 >= items[k][3]:
                    finish(items[k])
            if i < len(items):
                score_step(items[i])

    p_v.release()
    p_qk.release()

    # ---------------- o_proj (partial against this core's Wo cols) ----
    # weight-stationary: lhsT = Wo chunk reused across all 4 q-chunks.
    # Output is produced TRANSPOSED ([D_out, S]); the host transposes back.
    with tc.tile_pool(name="oev", bufs=4) as oevp, \
         tc.tile_pool(name="po", bufs=1, space="PSUM") as pop:
        for dt in range(D // P):
            po = [pop.tile([P, 512], F32, name=f"po{dt}_{qc}", tag=f"po{qc}", bufs=2)
                  for qc in range(NQ)]
            for h in range(H):
                for qc in range(NQ):
                    nc.tensor.matmul(po[qc][:], _r(wo_sb[:, h, dt * P:(dt + 1) * P]),
                                     _r(oT[:, h, qc * 512:(qc + 1) * 512]),
                                     start=(h == 0), stop=(h == H - 1))
            for qc in range(NQ):
                ot = oevp.tile([P, 512], BF16, name=f"ot{dt}_{qc}", tag="ot")
                if (dt + qc) % 2 == 0:
                    nc.vector.tensor_copy(ot[:], po[qc][:])
                else:
                    nc.scalar.activation(ot[:], po[qc][:], AFT.Copy)
                nc.sync.dma_start(out[dt * P:(dt + 1) * P, qc * 512:(qc + 1) * 512], ot[:])
    p_wo.release()
    p_oT.release()


def build_nc():
    nc = bacc.Bacc("TRN2", target_bir_lowering=False, debug=False,
                   enable_asserts=True, num_devices=8)
    xT = nc.dram_tensor("xT", [D, S], F32R, kind="ExternalInput").ap()
    wqT = nc.dram_tensor("wqT", [D, F], F32R, kind="ExternalInput").ap()
    wkT = nc.dram_tensor("wkT", [D, F], F32R, kind="ExternalInput").ap()
    wvT = nc.dram_tensor("wvT", [D, F], F32R, kind="ExternalInput").ap()
    woT = nc.dram_tensor("woT", [F, D], F32R, kind="ExternalInput").ap()
    cosT = nc.dram_tensor("cosT", [P, S], F32, kind="ExternalInput").ap()
    sinT = nc.dram_tensor("sinT", [P, S], F32, kind="ExternalInput").ap()
    mskT = nc.dram_tensor("mskT", [P, P], BF16, kind="ExternalInput").ap()
    out = nc.dram_tensor("out", [S, D], BF16, kind="ExternalOutput").ap()

    with tile.TileContext(nc) as tc:
        _body(tc, xT, wqT, wkT, wvT, woT, cosT, sinT, mskT, out)
    nc.compile()
    return nc


_CACHE = {}


def _get_nc():
    if "nc" not in _CACHE:
        _CACHE["nc"] = build_nc()
    return _CACHE["nc"]


def _rope_tables():
    hd = HD
    inv = 1.0 / (10000.0 ** (np.arange(0, hd, 2, dtype=np.float32) / np.float32(hd)))
    t = np.arange(S, dtype=np.float32)
    freqs = np.outer(t, inv)                      # [S, 64]
    emb = np.concatenate([freqs, freqs], axis=-1)  # [S, 128]
    cosT = np.cos(emb).T.astype(np.float32).copy()
    sinT = np.sin(emb).T.astype(np.float32).copy()
    sinT[0:64, :] *= -1.0  # sign of rotate_half baked into the table
    return np.ascontiguousarray(cosT), np.ascontiguousarray(sinT)


def _diag_masks():
    import ml_dtypes
    kp = np.arange(P)[:, None]
    qf = np.arange(P)[None, :]
    return np.ascontiguousarray((kp <= qf).astype(ml_dtypes.bfloat16))


def _in_maps(x, Wq, Wk, Wv, Wo):
    cosT, sinT = _rope_tables()
    msk = _diag_masks()
    maps = []
    for c in range(8):
        b, g = c // 4, c % 4
        fs = slice(g * F, (g + 1) * F)
        maps.append({
            "xT": np.ascontiguousarray(x[b].T),
            "wqT": np.ascontiguousarray(Wq[fs, :].T),
            "wkT": np.ascontiguousarray(Wk[fs, :].T),
            "wvT": np.ascontiguousarray(Wv[fs, :].T),
            "woT": np.ascontiguousarray(Wo[:, fs].T),
            "cosT": cosT,
            "sinT": sinT,
            "mskT": msk,
        })
    return maps


def run(x, Wq, Wk, Wv, Wo, trace=False, **spmd_kwargs):
    """Run on 8 cores; returns (full_output, BassKernelResults)."""
    x = np.asarray(x, np.float32)
    Wq = np.asarray(Wq, np.float32)
    Wk = np.asarray(Wk, np.float32)
    Wv = np.asarray(Wv, np.float32)
    Wo = np.asarray(Wo, np.float32)
    nc = _get_nc()
    maps = _in_maps(x, Wq, Wk, Wv, Wo)
    res = bass_utils.run_bass_kernel_spmd(nc, maps, core_ids=list(range(8)),
                                          trace=trace, **spmd_kwargs)
    outs = [np.asarray(res.results[c]["out"], dtype=np.float32) for c in range(8)]
    full = np.empty((2, S, D), np.float32)
    for b in range(2):
        # each core returns its o_proj partial TRANSPOSED ([D_out, S]), bf16
        acc = outs[4 * b] + outs[4 * b + 1] + outs[4 * b + 2] + outs[4 * b + 3]
        full[b] = acc.T
    return full, res


def kernel(x, Wq, Wk, Wv, Wo):
    full, _ = run(x, Wq, Wk, Wv, Wo)
    return full



# revision 29
# speedup vs baseline: 1.1886x; 1.0048x over previous
"""Multi-head attention (RoPE, causal, fp32) on 8 Trainium2 NeuronCores.

Problem: B=2, S=2048, D=2048, H=16 heads (hd=128).
Sharding: DP=2 (batch) x TP=4 (head groups of 4 heads). Core c handles
batch c//4, head group c%4. Each core computes q/k/v projections for its
512 features, RoPE, causal attention, and a partial o_proj against its
512 columns of Wo. The host sums the 4 partial o_proj outputs per batch.

Kernel layout strategy (per core):
  - qT, kT in [hd, seq] ("transposed") layout straight out of the
    projection matmuls; v in natural [seq, feat] layout. RoPE applied in
    place at eviction time (rowswap via SBUF->SBUF DMA, sign baked into
    the host-provided sin table).
  - Attention entirely in transposed space: scoresT[k, q] tiles from
    lhsT=kT slice, rhs=qT chunk, N=512. exp fused into the PSUM
    eviction on ScalarE (scale=1/sqrt(hd)), software-pipelined with the
    denominator / attn@V accumulation matmuls two steps behind.
    Causal: only j <= q k-tiles are computed; on the diagonal tile the
    fully-masked 128-wide sub-blocks are zeroed and a single shared
    [128,128] triangular mask is multiplied in.
    Softmax denominator via an all-ones [128,128] stationary matmul
    (yields the k-sum pre-broadcast across partitions); 1/denom via
    4 split VectorE reciprocals; normalization folded into the attn@V
    PSUM eviction.
  - o_proj is weight-stationary and emits the partial TRANSPOSED
    ([D_out, S]); the host transposes back while summing the 4 per-batch
    partials.
All matmuls run as float32r (fp32_mode=HIGH single-pass: full column
rate on fp32 data, ~1e-4-grade precision). Producers of f32r-consumed
SBUF data must write f32r-rounded outputs (BIR verifier requirement).
"""

import sys

for _p in ("/opt/trn_rl_repo",):
    if _p not in sys.path:
        sys.path.insert(0, _p)

import numpy as np

import concourse.bass as bass
import concourse.mybir as mybir
import concourse.tile as tile
from concourse import bacc, bass_utils


def _enable_ldw_opt():
    """walrus ships with --enable-ldw-opt=false; turning it on lets codegen
    elide weight reloads for consecutive matmuls sharing a stationary
    operand (the o_proj and denominator matmuls rely on this)."""
    if getattr(bass_utils, "_ldw_opt_patched", False):
        return
    orig = bass_utils.run_command

    def patched(argv, **kw):
        argv = ["--enable-ldw-opt=true" if a == "--enable-ldw-opt=false" else a
                for a in argv]
        return orig(argv, **kw)

    bass_utils.run_command = patched
    bass_utils._ldw_opt_patched = True


# NOTE: ldw-opt stays OFF: walrus's CoreV3 codegen crashes on bf16
# LDWEIGHTS when --enable-ldw-opt=true, and the bf16 attention path is
# worth more than the elision (weight loads shadow-hide under the 512-wide
# matmuls anyway).
# _enable_ldw_opt()

P = 128          # partitions / head dim
S = 2048         # sequence length
D = 2048         # model dim
F = 512          # features per core (4 heads)
H = 4            # heads per core
HD = 128         # head dim
NJ = D // P      # 16 contraction chunks of 128
NQ = S // 512    # 4 query chunks of 512
SCALE = 1.0 / float(np.sqrt(HD))

F32 = mybir.dt.float32
F32R = mybir.dt.float32r
BF16 = mybir.dt.bfloat16
AFT = mybir.ActivationFunctionType


def _r(ap):
    """View an fp32 AP as float32r for full-rate PE matmuls."""
    return ap.bitcast(F32R)


def _body(tc, xT, wqT, wkT, wvT, woT, cosT, sinT, mskT, out):
    nc = tc.nc

    # long-lived slabs with hand-managed lifetimes; pools are per-side LIFO
    # stacks, so the q/k/v slabs live on the "left" stack while phase-local
    # pools and the oT slab (which outlives q/k/v) use the default side.
    p_qk = tc.alloc_tile_pool(name="p_qk", bufs=1, side="left")   # phases 1..3
    qT = p_qk.tile([P, H, S], F32)    # [hd, head, seq]
    kT = p_qk.tile([P, H, S], F32)

    # v, the exp'd attention weights, and the final output travel as bf16:
    # same PE matmul rate, but half the ScalarE exp time (the attention
    # pace-setter), half the DVE mask time, and half the output DMA. The
    # rel-err budget (2e-2) dwarfs the ~1e-3 this costs.
    p_v = tc.alloc_tile_pool(name="p_v", bufs=1, side="left")     # phases 1..3
    vN = p_v.tile([P, NJ, F], BF16)  # [:, j, :] = v[j*128:(j+1)*128, :]

    # ---------------- projections: q, k (transposed layout) + RoPE ----
    p_xs = tc.alloc_tile_pool(name="p_xs", bufs=12, side="left")   # phases 1..2
    # cs/rot live on the LEFT stack: when the phase-1 pools close, the right
    # stack must free down to wqk only, so p_wv's loads WAR only against the
    # qk matmuls (not the RoPE tail that reads cos/sin/rot).
    with tc.tile_pool(name="cs", bufs=1, side="left") as cspool, \
         tc.tile_pool(name="wqk", bufs=1) as wpool, \
         tc.tile_pool(name="rot", bufs=1, side="left") as rpool, \
         tc.tile_pool(name="pp", bufs=1, space="PSUM") as pp:
        cos_sb = cspool.tile([P, 1, S], F32)
        sin_sb = cspool.tile([P, 1, S], F32)
        wq_sb = wpool.tile([P, NJ, F], F32R)
        wk_sb = wpool.tile([P, NJ, F], F32R)
        for s in range(NQ):
            pq = [pp.tile([P, 512], F32, name=f"pq{s}_{h}", tag=f"pq{h}")
                  for h in range(H)]
            pk = [pp.tile([P, 512], F32, name=f"pk{s}_{h}", tag=f"pk{h}")
                  for h in range(H)]
            for j in range(NJ):
                xt = p_xs.tile([P, 512], F32R, name=f"xt{s}_{j}", tag="xt")
                # x chunk first: the very first matmul waits on it, the
                # weight chunks follow right behind in the same queues
                nc.sync.dma_start(xt[:], xT[j * P:(j + 1) * P, s * 512:(s + 1) * 512])
                if s == 0:
                    # weight loads interleaved with the first x chunks so
                    # the first matmuls aren't stuck behind 8MB of DMA
                    nc.sync.dma_start(wq_sb[:, j, :], wqT[j * P:(j + 1) * P, :])
                    nc.sync.dma_start(wk_sb[:, j, :], wkT[j * P:(j + 1) * P, :])

                for h in range(H):
                    nc.tensor.matmul(pq[h][:], _r(wq_sb[:, j, h * HD:(h + 1) * HD]),
                                     _r(xt[:]), start=(j == 0), stop=(j == NJ - 1))
                    nc.tensor.matmul(pk[h][:], _r(wk_sb[:, j, h * HD:(h + 1) * HD]),
                                     _r(xt[:]), start=(j == 0), stop=(j == NJ - 1))
            sl = slice(s * 512, (s + 1) * 512)
            # cos/sin arrive piecewise, issued here (between chunks) so the
            # 256KB transfers never queue ahead of the per-j x-tile loads
            nc.sync.dma_start(cos_sb[:, 0, sl], cosT[:, sl])
            nc.sync.dma_start(sin_sb[:, 0, sl], sinT[:, sl])
            # Per head: evict the tile plus a rowswapped copy straight out
            # of PSUM. The swap is done by evicting the two partition halves
            # crosswise (engines can read/write different partition
            # windows) -- no SBUF->SBUF DMAs, so the x/weight loads never
            # queue behind the RoPE tail. Full evictions on ScalarE; the
            # half evictions split ScalarE/VectorE so the 24-op chain drains
            # ~2x faster. pq tags evicted before pk: the v-projection (which
            # reuses the pq banks) can then start after only 4 tags free.
            rts = {}
            for psl, slab, tag in ((pq, qT, "rtq"), (pk, kT, "rtk")):
                rt = rpool.tile([P, H, 512], F32, name=f"rt{s}_{tag}", tag=tag)
                rts[tag] = rt
                for h in range(H):
                    if h == 0:
                        # head 0's three reads split across both engines:
                        # its PSUM bank (the first one the next phase
                        # touches) frees in ~800ns instead of ~1.3us
                        nc.vector.tensor_copy(slab[:, h, sl].bitcast(F32R),
                                              psl[h][:])
                        nc.scalar.activation(rt[0:64, h, :], psl[h][64:128, :],
                                             AFT.Copy)
                        nc.scalar.activation(rt[64:128, h, :], psl[h][0:64, :],
                                             AFT.Copy)
                        continue
                    nc.scalar.activation(slab[:, h, sl].bitcast(F32R), psl[h][:],
                                         AFT.Copy)
                    nc.scalar.activation(rt[0:64, h, :], psl[h][64:128, :], AFT.Copy)
                    nc.vector.tensor_copy(rt[64:128, h, :], psl[h][0:64, :])
            # ...then RoPE in place, 4 heads per DVE op:
            # dst = dst*cos + rowswap(dst)*sin (sign baked into sin table)
            for slab, tag in ((qT, "rtq"), (kT, "rtk")):
                rt = rts[tag]
                dst = slab[:, :, sl]
                _, cos_b = bass.broadcast_tensor_aps(dst, cos_sb[:, :, sl])
                _, sin_b = bass.broadcast_tensor_aps(dst, sin_sb[:, :, sl])
                nc.vector.tensor_mul(rt[:], rt[:], sin_b)
                nc.vector.tensor_mul(dst.bitcast(F32R), dst, cos_b)
                nc.vector.tensor_add(dst.bitcast(F32R), dst, rt[:])

        # ------------ projection: v (natural layout), same pools ----------
        # wv reuses the wq_sb slab (its readers -- the chunk-3 matmuls --
        # finish progressively, so these loads prefetch during chunk 3) and
        # the pv accumulators reuse the pq/pk PSUM tags. No new pools means
        # no pool-boundary barrier: the old structure serialized v behind
        # the whole RoPE tail via the PSUM pool alloc.
        for j in range(NJ):
            nc.sync.dma_start(wq_sb[:, j, :], wvT[j * P:(j + 1) * P, :])
        for sg in range(4):
            tg = "pq" if sg % 2 == 0 else "pk"
            pv = [pp.tile([P, F], F32, name=f"pv{sg}_{st}", tag=f"{tg}{st}")
                  for st in range(4)]
            for j in range(NJ):
                xt2 = p_xs.tile([P, 512], F32R, name=f"x2{sg}_{j}", tag="xt")
                nc.sync.dma_start(xt2[:], xT[j * P:(j + 1) * P, sg * 512:(sg + 1) * 512])
                for st in range(4):
                    nc.tensor.matmul(pv[st][:], _r(xt2[:, st * P:(st + 1) * P]),
                                     _r(wq_sb[:, j, :]), start=(j == 0), stop=(j == NJ - 1))
            for st in range(4):
                if st % 2 == 0:
                    nc.scalar.activation(vN[:, sg * 4 + st, :], pv[st][:], AFT.Copy)
                else:
                    nc.vector.tensor_copy(vN[:, sg * 4 + st, :], pv[st][:])

    p_xs.release()

    # ---------------- attention (all in transposed space) -------------
    # attention-phase SBUF lives on the RIGHT stack so nothing here reuses
    # the just-released wv/xs2 space (which would add WAR waits on the tail
    # of the v pass). Wo is prefetched here too, for the same reason plus
    # DMA overlap with attention compute.
    p_oT = tc.alloc_tile_pool(name="p_oT", bufs=1, side="right")  # phases 3..4
    oT = p_oT.tile([P, H, S], F32)    # attention output, transposed
    # Wo prefetched below the attention pools on the right stack so its 4MB
    # of DMA overlaps attention compute and o_proj starts without a stall.
    p_wo = tc.alloc_tile_pool(name="p_wo", bufs=1, side="right")
    wo_sb = p_wo.tile([P, H, D], F32R)
    with tc.tile_pool(name="amsk", bufs=1, side="right") as mpool, \
         tc.tile_pool(name="exp", bufs=1, side="right") as epool, \
         tc.tile_pool(name="attsb", bufs=2, side="right") as apool, \
         tc.tile_pool(name="pa", bufs=1, space="PSUM") as pap:
        msk_sb = mpool.tile([P, P], BF16)
        nc.sync.dma_start(msk_sb[:], mskT)
        for h in range(H):
            nc.sync.dma_start(wo_sb[:, h, :], woT[h * P:(h + 1) * P, :])
        # all-ones [128,128] stationary: the denominator matmul then yields
        # the k-sum already broadcast across all 128 partitions of PSUM.
        ones_tmp = mpool.tile([P, P], BF16)
        nc.vector.memset(ones_tmp[:], 1.0)
        ones_mat = mpool.tile([P, P], BF16)
        nc.vector.tensor_copy(ones_mat[:], ones_tmp[:])

        # One flat software pipeline over all (head, q-chunk, k-pair) work
        # items: the accumulation matmuls lag the score/exp stage by LAG
        # pairs and cross (h,q) boundaries, so the exp latency is never
        # exposed at iteration starts. Within a step the acc is issued
        # BEFORE the next score so reuse of the shared ex buffer is a
        # plain engine-ordered WAR, never a corruption.
        # Causal narrowing: k-tile j only attends queries >= 128*(j-4q), so
        # score/exp/acc all operate on the [128*d, 512) column slice; the
        # fully-masked sub-blocks are never computed at all.
        items = []
        for h in range(H):
            for q in range(NQ):
                jmax = 4 * (q + 1)
                for j in range(0, jmax, 2):
                    items.append((h, q, j, jmax))
        state = {}

        def _lo(q, j):
            # fp32r matmuls below 256 moving-dim run at 1/4 rate, so the
            # f32r score matmuls clamp the causal narrowing at 256 columns
            return max(0, min((j - 4 * q), 2)) * P

        def _alo(q, j):
            # the accumulation matmuls have bf16 rhs (no sub-256 penalty),
            # so they narrow fully to the true causal boundary; the exp'd
            # scores in the clamp slack [256:384) are then simply never
            # read, and no zero-fill is needed
            return max(0, j - 4 * q) * P

        def score_step(it):
            h, q, j, jmax = it
            key = (h, q)
            if key not in state:
                # double-buffered across (h,q) iterations: the next
                # iteration's exp never WAR-waits on this one's last accs
                state[key] = {"ex": epool.tile([P, NJ, 512], BF16,
                                               name=f"ex{h}_{q}",
                                               tag=f"ex{(h * NQ + q) % 2}")}
            ex = state[key]["ex"]
            lo = _lo(q, j)
            psc = pap.tile([P, 2, 512], F32, name=f"psc{h}{q}{j}",
                           tag="psc", bufs=2)
            for t in range(2):
                tlo = _lo(q, j + t)
                nc.tensor.matmul(psc[:, t, tlo:512],
                                 _r(kT[:, h, (j + t) * P:(j + t + 1) * P]),
                                 _r(qT[:, h, q * 512 + tlo:(q + 1) * 512]),
                                 start=True, stop=True)
            nc.scalar.activation(ex[:, j:j + 2, lo:512],
                                 psc[:, :, lo:512], AFT.Exp, scale=SCALE)
            for t in range(2):
                dt = (j + t) - 4 * q
                if dt >= 0:
                    nc.vector.tensor_mul(
                        ex[:, j + t, dt * P:(dt + 1) * P],
                        ex[:, j + t, dt * P:(dt + 1) * P], msk_sb[:])
                if dt == 3:
                    # clamped block [256:384) holds exp of fully-masked
                    # scores; zero it before the accumulation reads it
                    nc.vector.tensor_scalar_mul(
                        ex[:, j + t, 2 * P:3 * P],
                        ex[:, j + t, 2 * P:3 * P], 0.0)

        def acc_pair(it):
            h, q, j, jmax = it
            st = state[(h, q)]
            ex = st["ex"]
            if "pden" not in st:
                st["pden"] = pap.tile([P, 512], F32, name=f"pden{h}{q}",
                                      tag="pden", bufs=2)
                st["pov"] = pap.tile([P, 512], F32, name=f"pov{h}{q}",
                                     tag="pov", bufs=2)
            pden, pov = st["pden"], st["pov"]
            lo = [_lo(q, j + t) for t in range(2)]
            # both pden matmuls back to back: ldw-opt elides the ones
            # reload on the second
            for t in range(2):
                nc.tensor.matmul(pden[:, lo[t]:512], ones_mat[:],
                                 ex[:, j + t, lo[t]:512],
                                 start=(j + t == 0), stop=(j + t == jmax - 1))
            for t in range(2):
                nc.tensor.matmul(pov[:, lo[t]:512],
                                 vN[:, j + t, h * HD:(h + 1) * HD],
                                 ex[:, j + t, lo[t]:512],
                                 start=(j + t == 0), stop=(j + t == jmax - 1))

        def finish(it):
            h, q, j, jmax = it
            st = state.pop((h, q))
            rbc = apool.tile([P, 512], F32, name=f"rbc{h}{q}", tag="rbc")
            nc.vector.reciprocal_approx_fast(rbc[:], st["pden"][:])
            nc.vector.tensor_mul(oT[:, h, q * 512:(q + 1) * 512].bitcast(F32R),
                                 st["pov"][:], rbc[:])

        LAG = 3
        for i in range(len(items) + LAG):
            k = i - LAG
            if k >= 0:
                acc_pair(items[k])
                if items[k][2] + 2 >= items[k][3]:
                    finish(items[k])
            if i < len(items):
                score_step(items[i])

        # ------------ o_proj (partial against this core's Wo cols) ------
        # weight-stationary: lhsT = Wo chunk reused across all 4 q-chunks.
        # Output is produced TRANSPOSED ([D_out, S]); the host transposes
        # back. Runs inside the attention pools, reusing the pden/pov/psc
        # PSUM tags (no pool boundary = no wait on the attention DVE tail);
        # consecutive dt iterations land on alternating tag slots, so the
        # accumulators stay fully double-buffered.
        for dt in range(D // P):
            po0 = pap.tile([P, 512], F32, name=f"po{dt}_0", tag="pden", bufs=2)
            po1 = pap.tile([P, 512], F32, name=f"po{dt}_1", tag="pov", bufs=2)
            po23 = pap.tile([P, 2, 512], F32, name=f"po{dt}_23", tag="psc", bufs=2)
            po = [po0[:], po1[:], po23[:, 0, :], po23[:, 1, :]]
            for h in range(H):
                for qc in range(NQ):
                    nc.tensor.matmul(po[qc], _r(wo_sb[:, h, dt * P:(dt + 1) * P]),
                                     _r(oT[:, h, qc * 512:(qc + 1) * 512]),
                                     start=(h == 0), stop=(h == H - 1))
            for qc in range(NQ):
                ot = apool.tile([P, 512], BF16, name=f"ot{dt}_{qc}", tag=f"ot{qc % 2}")
                if (dt + qc) % 2 == 0:
                    nc.vector.tensor_copy(ot[:], po[qc])
                else:
                    nc.scalar.activation(ot[:], po[qc], AFT.Copy)
                nc.sync.dma_start(out[dt * P:(dt + 1) * P, qc * 512:(qc + 1) * 512], ot[:])

    p_v.release()
    p_qk.release()
    p_wo.release()
    p_oT.release()


def build_nc():
    nc = bacc.Bacc("TRN2", target_bir_lowering=False, debug=False,
                   enable_asserts=True, num_devices=8)
    xT = nc.dram_tensor("xT", [D, S], F32R, kind="ExternalInput").ap()
    wqT = nc.dram_tensor("wqT", [D, F], F32R, kind="ExternalInput").ap()
    wkT = nc.dram_tensor("wkT", [D, F], F32R, kind="ExternalInput").ap()
    wvT = nc.dram_tensor("wvT", [D, F], F32R, kind="ExternalInput").ap()
    woT = nc.dram_tensor("woT", [F, D], F32R, kind="ExternalInput").ap()
    cosT = nc.dram_tensor("cosT", [P, S], F32, kind="ExternalInput").ap()
    sinT = nc.dram_tensor("sinT", [P, S], F32, kind="ExternalInput").ap()
    mskT = nc.dram_tensor("mskT", [P, P], BF16, kind="ExternalInput").ap()
    out = nc.dram_tensor("out", [S, D], BF16, kind="ExternalOutput").ap()

    with tile.TileContext(nc) as tc:
        _body(tc, xT, wqT, wkT, wvT, woT, cosT, sinT, mskT, out)
    nc.compile()
    return nc


_CACHE = {}


def _get_nc():
    if "nc" not in _CACHE:
        _CACHE["nc"] = build_nc()
    return _CACHE["nc"]


def _rope_tables():
    hd = HD
    inv = 1.0 / (10000.0 ** (np.arange(0, hd, 2, dtype=np.float32) / np.float32(hd)))
    t = np.arange(S, dtype=np.float32)
    freqs = np.outer(t, inv)                      # [S, 64]
    emb = np.concatenate([freqs, freqs], axis=-1)  # [S, 128]
    cosT = np.cos(emb).T.astype(np.float32).copy()
    sinT = np.sin(emb).T.astype(np.float32).copy()
    sinT[0:64, :] *= -1.0  # sign of rotate_half baked into the table
    return np.ascontiguousarray(cosT), np.ascontiguousarray(sinT)


def _diag_masks():
    import ml_dtypes
    kp = np.arange(P)[:, None]
    qf = np.arange(P)[None, :]
    return np.ascontiguousarray((kp <= qf).astype(ml_dtypes.bfloat16))


def _in_maps(x, Wq, Wk, Wv, Wo):
    cosT, sinT = _rope_tables()
    msk = _diag_masks()
    maps = []
    for c in range(8):
        b, g = c // 4, c % 4
        fs = slice(g * F, (g + 1) * F)
        maps.append({
            "xT": np.ascontiguousarray(x[b].T),
            "wqT": np.ascontiguousarray(Wq[fs, :].T),
            "wkT": np.ascontiguousarray(Wk[fs, :].T),
            "wvT": np.ascontiguousarray(Wv[fs, :].T),
            "woT": np.ascontiguousarray(Wo[:, fs].T),
            "cosT": cosT,
            "sinT": sinT,
            "mskT": msk,
        })
    return maps


def run(x, Wq, Wk, Wv, Wo, trace=False, **spmd_kwargs):
    """Run on 8 cores; returns (full_output, BassKernelResults)."""
    x = np.asarray(x, np.float32)
    Wq = np.asarray(Wq, np.float32)
    Wk = np.asarray(Wk, np.float32)
    Wv = np.asarray(Wv, np.float32)
    Wo = np.asarray(Wo, np.float32)
    nc = _get_nc()
    maps = _in_maps(x, Wq, Wk, Wv, Wo)
    res = bass_utils.run_bass_kernel_spmd(nc, maps, core_ids=list(range(8)),
                                          trace=trace, **spmd_kwargs)
    outs = [np.asarray(res.results[c]["out"], dtype=np.float32) for c in range(8)]
    full = np.empty((2, S, D), np.float32)
    for b in range(2):
        # each core returns its o_proj partial TRANSPOSED ([D_out, S]), bf16
        acc = outs[4 * b] + outs[4 * b + 1] + outs[4 * b + 2] + outs[4 * b + 3]
        full[b] = acc.T
    return full, res


def kernel(x, Wq, Wk, Wv, Wo):
    full, _ = run(x, Wq, Wk, Wv, Wo)
    return full

